# revision 22
# baseline (speedup 1.0000x reference)
"""GQA attention block (B=2, S=2048, H=2048, NH=16, NKV=4, HD=128) on 8 TRN2
NeuronCores.

Sharding: 2 batch groups x 4-way tensor parallel over heads.
Core c = b*4 + l handles batch b, q-heads [4l, 4l+4), kv-head l, and computes
output rows [512l, 512(l+1)) of y[b]^T after per-(head, i-half) AllGathers of
the context over its 4-core group. The host passes x and all weights
pre-transposed/pre-tiled (pure layout; bf16 where unit-scale) so the device
does no layout work on x/W at all. The output is produced transposed
([OSL, S]); the host transposes back for free at unshard time.

Per-core device pipeline (PE-queue kept gapless; consecutive matmuls share
lhsT wherever possible to skip the ~128-row weight reload):
  phase 1: three passes (k,v), (q0,q1), (q2,q3) over resident x tiles; each
           weight's 4x N=512 matmuls run back-to-back (one reload per weight
           per chunk). Epilogues add position bias split across DVE/Pool
           halves; vT staged bf16 via ACT.
  phase 2: isl-major software pipeline over 16 (islice, head) blocks; per
           128-query slot: S in two 2-bank PSUM halves (4x N=512 f32r sharing
           the qt lhsT), row-max from a stride-16 subset of the second half
           only (safe: margin analysis in-line), two exps with fused bias +
           row-sum (ACT, bf16 out), deferred recip+normalize (DVE; the last
           slot's normalize is quarter-split so it never gates the PE).
           The previous block's A.T transposes and A@V (bf16, N=512) are
           emitted alternating after each slot with a one-quarter lag so the
           PE never waits on the Pool AT copybacks.
  comm:    AllGather of ctx.T (bf16) per (head, i-half) (8 x 0.5MB), fired
           as soon as both islices of the half are written; gathered halves
           are pulled on-chip immediately (sync queue).
  phase 3: y^T[o, i] = sum_hd woT[hd, o] ctx^T[hd, i]; per (o-chunk, hd-chunk)
           the f0/f1 half matmuls share the wo lhsT. Heads are accumulated in
           AllGather-completion order so the last AG hides under earlier work.

Numerics: f32r (~1e-4) for q/k + scores keeps the huge position-bias score
component (pos_i*pos_j*HD ~ 4.7e3 in logits) accurate; the k-side bias is
centered (c'_j = 0.01*pos_j - mid, softmax-invariant) to halve its rounding
noise; bf16 only where unit-scale (QKV inputs, attention weights, ctx, Wo).
Subset-max margin ('ones' mask): positions are nondecreasing, so the row max
sits in the top i-half up to the +-51-ish data spread (c'*|q.1|/sqrt(HD));
stride-16 with offset 15 includes j=S-1, leaving exp(S-m) <= ~e^63 and row
sums <= ~e^71, both comfortably inside f32/bf16 range.
"""
import numpy as np

import concourse.bass as bass
import concourse.mybir as mybir
from concourse import bacc, tile
from concourse.bass_utils import run_bass_kernel_spmd

import ml_dtypes

F32 = mybir.dt.float32
F32R = mybir.dt.float32r
BF16 = mybir.dt.bfloat16
AF = mybir.ActivationFunctionType
ALU = mybir.AluOpType

B, S, H = 2, 2048, 2048
NH, NKV, HD = 16, 4, 128
TP = 4                      # tensor-parallel group size
QH = NH // TP               # q heads per core (4)
OSL = H // TP               # output rows per core (512)
SCALE = 1.0 / np.sqrt(HD)
NHC = H // 128              # 16 contraction chunks of 128
NIT = S // 128              # 16 i-tiles
NISL = S // 512             # 4 i-slices of 512
SH = S // 2                 # i-half (1024)

_CACHED = {}


def _build(mask_mode):
    """mask_mode: 'ones' (ignore mask) or 'binary' (additive -1e9 bias)."""
    nc = bacc.Bacc("TRN2", target_bir_lowering=False, debug=False, num_devices=8)

    xt = nc.dram_tensor("xt", [H, S], BF16, kind="ExternalInput")
    wqt = nc.dram_tensor("wqt", [128, QH, NHC, HD], BF16, kind="ExternalInput")
    wkt = nc.dram_tensor("wkt", [128, NHC, HD], BF16, kind="ExternalInput")
    wvt = nc.dram_tensor("wvt", [128, NHC, HD], BF16, kind="ExternalInput")
    wot = nc.dram_tensor("wot", [128, NHC, OSL], BF16, kind="ExternalInput")
    posq = nc.dram_tensor("posq", [128, S], F32, kind="ExternalInput")
    posk = nc.dram_tensor("posk", [128, S], F32, kind="ExternalInput")
    maskb = nc.dram_tensor("maskb", [1, S], F32, kind="ExternalInput")
    out = nc.dram_tensor("out", [OSL, S], F32, kind="ExternalOutput")

    idb_dram = nc.inline_tensor(np.eye(128, dtype=ml_dtypes.bfloat16), name="idb")

    groups = [[0, 1, 2, 3], [4, 5, 6, 7]]

    with tile.TileContext(nc) as tc:
        with (
            tc.tile_pool(name="pers", bufs=1) as pers,
            tc.tile_pool(name="small", bufs=14) as small,
            tc.tile_pool(name="dram", bufs=1, space="DRAM") as dram,
        ):
            # ---------------- persistent tiles ----------------
            qt_sb = pers.tile([128, QH, S], F32R)       # [d, h, i]  32KB/part
            kt_sb = pers.tile([128, S], F32R)           # [d, j]      8KB/part
            v_sb = pers.tile([128, NHC, HD], BF16)      # [j, jc, d]  4KB/part
            wo_sb = pers.tile([128, NHC, OSL], BF16)    # 16KB/part
            idb_sb = pers.tile([128, 128], BF16)

            # AG bounce buffers (per head, per i-half)
            cin = [[dram.tile([128, SH], BF16, name=f"cin{h}_{f}")
                    for f in range(2)] for h in range(QH)]
            gout = [[dram.tile([TP * 128, SH], BF16, name=f"gout{h}_{f}")
                     for f in range(2)] for h in range(QH)]

            # ---------------- phase 1: QKV projections ----------------
            with tc.tile_pool(name="p1w", bufs=1) as p1w:
                xt_sb = p1w.tile([128, NHC, S], BF16)    # resident x, 64KB/part
                wq_sb = p1w.tile([128, QH, NHC, HD], BF16)
                wk_sb = p1w.tile([128, NHC, HD], BF16)
                wv_sb = p1w.tile([128, NHC, HD], BF16)
                posq_sb = p1w.tile([128, S], F32)
                posk_sb = p1w.tile([128, S], F32)
                vt_stage = p1w.tile([128, S], BF16)      # vT [d, j] staged

                # pass-1 gates on only wk+wv+xt[0] (~1.6MB); wq/pos/wo are
                # dispatched after the pass-1 matmul emission so their
                # transfers don't steal HBM bandwidth from the gating loads.
                nc.scalar.dma_start(wk_sb[:], wkt[:])
                nc.scalar.dma_start(wv_sb[:], wvt[:])
                for hc in range(NHC):
                    nc.sync.dma_start(xt_sb[:, hc, :],
                                      xt[hc * 128:(hc + 1) * 128, :])
                nc.sync.dma_start(idb_sb[:], idb_dram.ap())

                def epi_add(dst, src, scalar, bias):
                    # DVE only: Pool/GpSimd cannot read PSUM on TRN2
                    for half in range(2):
                        sl = slice(half * SH, (half + 1) * SH)
                        nc.vector.scalar_tensor_tensor(
                            dst[:, sl], src[:, sl], scalar, bias[:, sl],
                            op0=ALU.mult, op1=ALU.add)

                with tc.tile_pool(name="p1ps", bufs=1, space="PSUM") as p1ps:
                    def pass_mms(wa, wb, pa, pb):
                        # matmul out is capped at one PSUM bank (512 f32):
                        # 4 slices back-to-back sharing the lhsT
                        for hc in range(NHC):
                            st, sp = hc == 0, hc == NHC - 1
                            for w_ap, pt in ((wa, pa), (wb, pb)):
                                for q4 in range(4):
                                    cs = slice(q4 * 512, (q4 + 1) * 512)
                                    nc.tensor.matmul(
                                        pt[:, cs], w_ap(hc), xt_sb[:, hc, cs],
                                        start=st, stop=sp,
                                        skip_group_check=True)

                    # pass 1: k, v
                    kp = p1ps.tile([128, S], F32, tag="pa", name="kp")
                    vp = p1ps.tile([128, S], F32, tag="pb", name="vp")
                    pass_mms(lambda hc: wk_sb[:, hc, :],
                             lambda hc: wv_sb[:, hc, :], kp, vp)
                    # non-gating loads dispatched once pass-1 owns the HBM
                    for hh in range(QH):
                        nc.scalar.dma_start(wq_sb[:, hh], wqt[:, hh])
                    nc.gpsimd.dma_start(posk_sb[:], posk[:])
                    nc.gpsimd.dma_start(posq_sb[:], posq[:])
                    epi_add(kt_sb, kp, 1.0, posk_sb)
                    nc.scalar.copy(vt_stage[:, 0:SH], vp[:, 0:SH])
                    nc.scalar.copy(vt_stage[:, SH:S], vp[:, SH:S])

                    # passes 2-3: q pairs
                    for pair in ((0, 1), (2, 3)):
                        qp = {hh: p1ps.tile([128, S], F32, tag=t,
                                            name=f"qp{hh}")
                              for hh, t in zip(pair, ("pa", "pb"))}
                        pass_mms(lambda hc, a=pair[0]: wq_sb[:, a, hc, :],
                                 lambda hc, b=pair[1]: wq_sb[:, b, hc, :],
                                 qp[pair[0]], qp[pair[1]])
                        for hh in pair:
                            epi_add(qt_sb[:, hh, :], qp[hh], SCALE, posq_sb)

            # mask bias (binary mode): additive row [1, S] f32r for accum-MM
            if mask_mode == "binary":
                maskb_sb = pers.tile([1, S], F32R)
                ones1_sb = pers.tile([1, 128], F32R)
                nc.gpsimd.dma_start(maskb_sb[:], maskb[:])
                ones_dram = nc.inline_tensor(
                    np.ones((1, 128), dtype=np.float32), name="ones1")
                nc.gpsimd.dma_start(ones1_sb[:], ones_dram.ap())

            # wo prefetch (phase 3), behind pos on the same queue
            nc.gpsimd.dma_start(wo_sb[:], wot[:])

            # ---------------- phase 2: attention ----------------
            cfp_cm = tc.tile_pool(name="cfp", bufs=1)
            cfp = cfp_cm.__enter__()
            ctxf = [[cfp.tile([128, TP, SH], BF16, name=f"ctxf{f}_{a}")
                     for f in range(2)] for a in range(QH)]  # 64KB/part
            with (
                tc.tile_pool(name="p2s", bufs=1, space="PSUM") as p2s,
                tc.tile_pool(name="p2tp", bufs=2, space="PSUM") as p2tp,
                tc.tile_pool(name="p2cx", bufs=2, space="PSUM") as p2cx,
                tc.tile_pool(name="p2a", bufs=8) as p2a,
                tc.tile_pool(name="p2at", bufs=3) as p2at,
                tc.tile_pool(name="p2ctx", bufs=1) as p2ctx,
            ):
                ctxT = {(h, f): p2ctx.tile([128, SH], BF16, name=f"cT{h}_{f}")
                        for h in range(QH) for f in range(2)}

                def scores_slot(blk, it4):
                    h, isl = blk["h"], blk["isl"]
                    it = isl * 4 + it4
                    Shs = [p2s.tile([128, SH], F32, tag=f"S{x}",
                                    name=f"S{x}_{h}_{it}") for x in range(2)]
                    q_ap = qt_sb[:, h, it * 128:(it + 1) * 128]
                    for js in range(4):
                        Sx = Shs[js // 2]
                        cs = slice((js % 2) * 512, (js % 2) * 512 + 512)
                        nc.tensor.matmul(
                            Sx[:, cs], q_ap, kt_sb[:, js * 512:(js + 1) * 512],
                            start=True, stop=(mask_mode != "binary"),
                            skip_group_check=True)
                        if mask_mode == "binary":
                            nc.tensor.matmul(
                                Sx[:, cs], ones1_sb[:],
                                maskb_sb[:, js * 512:(js + 1) * 512],
                                start=False, stop=True, skip_group_check=True)
                    negm = small.tile([128, 1], F32, tag="nm",
                                      name=f"nm_{h}_{it}")
                    if mask_mode == "binary":
                        # full-row subset max: either half may be masked out
                        n0 = small.tile([128, 1], F32, tag="n0", name=f"n0_{it}")
                        nc.vector.tensor_reduce(
                            n0[:], Shs[0][:, 15:SH:16],
                            axis=mybir.AxisListType.X, op=ALU.max, negate=True)
                        nc.vector.tensor_reduce(
                            negm[:], Shs[1][:, 15:SH:16],
                            axis=mybir.AxisListType.X, op=ALU.max, negate=True)
                        nc.vector.scalar_tensor_tensor(
                            negm[:], negm[:], 0.0, n0[:],
                            op0=ALU.add, op1=ALU.min)
                    else:
                        # top-half subset max only (margin analysis in module
                        # docstring: positions nondecreasing => safe shift)
                        nc.vector.tensor_reduce(
                            negm[:], Shs[1][:, 15:SH:16],
                            axis=mybir.AxisListType.X, op=ALU.max, negate=True)
                    A = p2a.tile([128, S], BF16, tag="A", name=f"A_{h}_{it}")
                    sums = [small.tile([128, 1], F32, tag=f"su{x}",
                                       name=f"su{x}_{h}_{it}") for x in range(2)]
                    for x in range(2):
                        nc.scalar.activation(
                            A[:, x * SH:(x + 1) * SH], Shs[x][:], AF.Exp,
                            bias=negm[:], scale=1.0, accum_out=sums[x][:])
                    return {"A": A, "sums": sums, "it": it, "r": None}

                def make_r(slot):
                    st = small.tile([128, 1], F32, tag="st",
                                    name=f"st_{slot['it']}")
                    r = small.tile([128, 1], F32, tag="r",
                                   name=f"r_{slot['it']}")
                    nc.vector.scalar_tensor_tensor(
                        st[:], slot["sums"][0][:], 0.0, slot["sums"][1][:],
                        op0=ALU.add, op1=ALU.add)
                    nc.vector.reciprocal(r[:], st[:])
                    slot["r"] = r

                def emit_norm_full(slot):
                    # DVE: bf16 in/out runs ~2.7 elem/cycle; GpSimd is a
                    # software DSP (~15 G elem/s) -- never bulk work there
                    make_r(slot)
                    nc.vector.tensor_scalar_mul(
                        slot["A"][:], slot["A"][:], slot["r"][:])

                def emit_norm3_quarter(blk, q):
                    # make_r(slot3) was emitted at the end of its own block,
                    # so only the quarter multiply sits before the T-group
                    slot = blk["slots"][3]
                    cs = slice(q * 512, (q + 1) * 512)
                    nc.vector.tensor_scalar_mul(
                        slot["A"][:, cs], slot["A"][:, cs], slot["r"][:])

                def emit_tgroup(blk, jc):
                    it4 = jc // 4
                    if jc % 4 == 0:
                        blk["atq"][it4] = p2at.tile(
                            [128, 4, 512], BF16, tag="AT",
                            name=f"AT{blk['bi']}_{it4}")
                    atq = blk["atq"][it4]
                    tp = p2tp.tile([128, 512], BF16, tag="tp",
                                   name=f"tp{blk['bi']}_{jc}")
                    for k in range(4):
                        nc.tensor.transpose(
                            tp[:, k * 128:(k + 1) * 128],
                            blk["slots"][k]["A"][:, jc * 128:(jc + 1) * 128],
                            idb_sb[:])
                    # PSUM->SBUF copybacks: 3 per slot on DVE, 1 on ACT
                    if jc % 4 == 0:
                        nc.scalar.copy(atq[:, jc % 4, :], tp[:])
                    else:
                        nc.vector.tensor_copy(atq[:, jc % 4, :], tp[:])

                def emit_av(blk, jc):
                    nc.tensor.matmul(
                        blk["ctxp"][:], v_sb[:, jc, :],
                        blk["atq"][jc // 4][:, jc % 4, :],
                        start=(jc == 0), stop=(jc == NHC - 1))
                    if jc == NHC - 1:
                        finish_block(blk)

                def finish_block(blk):
                    h, isl = blk["h"], blk["isl"]
                    f, seg = isl // 2, isl % 2
                    ct = ctxT[(h, f)]
                    nc.vector.tensor_copy(
                        ct[:, seg * 512:(seg + 1) * 512], blk["ctxp"][:])
                    if seg == 1:
                        nc.sync.dma_start(cin[h][f][:], ct[:])
                        nc.gpsimd.collective_compute(
                            "AllGather", ALU.bypass,
                            ins=[cin[h][f][:].opt()],
                            outs=[gout[h][f][:].opt()],
                            replica_groups=groups)
                        nc.sync.dma_start(
                            ctxf[h][f][:],
                            gout[h][f][:].rearrange("(lr p) i -> p lr i", p=128))

                blocks = [(isl, h) for isl in range(NISL) for h in range(QH)]
                prev = None
                tq = []        # pending (blk, jc) whose AV trails 4 T-groups
                vt_done = 0    # v-transpose groups used to fill block 0
                for bi, (isl, h) in enumerate(blocks):
                    blk = {"h": h, "isl": isl, "bi": bi, "slots": [],
                           "atq": {},
                           "ctxp": p2cx.tile([128, 512], F32, tag="cx",
                                             name=f"cx{bi}")}
                    for it4 in range(4):
                        if prev is not None:
                            emit_norm3_quarter(prev, it4)
                        blk["slots"].append(scores_slot(blk, it4))
                        if it4 >= 1:
                            emit_norm_full(blk["slots"][it4 - 1])
                        for k in range(4):
                            # AV first: its copyback landed ~4 groups ago, so
                            # it never waits; the T-group behind it gives the
                            # norm3 chain time at block starts
                            if len(tq) >= 4:
                                emit_av(*tq.pop(0))
                            if prev is not None:
                                emit_tgroup(prev, it4 * 4 + k)
                                tq.append((prev, it4 * 4 + k))
                            elif vt_done < 4 and k == 0:
                                # fill block 0: 4 v-transposes per slot
                                g = vt_done
                                tpv = p2tp.tile([128, 512], BF16, tag="tp",
                                                name=f"tpv{g}")
                                for kk in range(4):
                                    jc = g * 4 + kk
                                    nc.tensor.transpose(
                                        tpv[:, kk * 128:(kk + 1) * 128],
                                        vt_stage[:, jc * 128:(jc + 1) * 128],
                                        idb_sb[:])
                                nc.vector.tensor_copy(
                                    v_sb[:, g * 4:(g + 1) * 4, :], tpv[:])
                                vt_done += 1
                    make_r(blk["slots"][3])
                    prev = blk
                # drain the last block
                for it4 in range(4):
                    emit_norm3_quarter(prev, it4)
                    for k in range(4):
                        if len(tq) >= 4:
                            emit_av(*tq.pop(0))
                        emit_tgroup(prev, it4 * 4 + k)
                        tq.append((prev, it4 * 4 + k))
                while tq:
                    emit_av(*tq.pop(0))

            # ---------------- phase 3: output projection (y^T) ----------
            with (
                tc.tile_pool(name="p3y", bufs=2, space="PSUM") as p3y,
                tc.tile_pool(name="p3o", bufs=3) as p3o,
            ):
                # f0 pass first: its AllGathers completed mid-phase-2, so the
                # PE rolls straight in; the last f1 AllGather (fired at the
                # end of phase 2) hides under the ~30us of f0 work.
                for f in range(2):
                    for o in range(OSL // 128):
                        yp = p3y.tile([128, SH], F32, tag=f"y{o % 2}",
                                      name=f"yp{o}_{f}")
                        for cc in range(NHC):
                            a, lr = cc // 4, cc % 4
                            w_ap = wo_sb[:, cc, o * 128:(o + 1) * 128]
                            st, sp = cc == 0, cc == NHC - 1
                            for q2 in range(2):
                                cs = slice(q2 * 512, (q2 + 1) * 512)
                                nc.tensor.matmul(
                                    yp[:, cs], w_ap, ctxf[a][f][:, lr, cs],
                                    start=st, stop=sp, skip_group_check=True)
                        y_sb = p3o.tile([128, SH], F32, tag="y_sb",
                                        name=f"y_sb{o}_{f}")
                        if o % 2 == 0:
                            nc.vector.tensor_copy(y_sb[:], yp[:])
                        else:
                            nc.scalar.copy(y_sb[:], yp[:])
                        nc.sync.dma_start(
                            out[o * 128:(o + 1) * 128, f * SH:(f + 1) * SH],
                            y_sb[:])

            cfp_cm.__exit__(None, None, None)

    nc.compile()
    return nc


def _get_nc(mask_mode):
    if mask_mode not in _CACHED:
        _CACHED[mask_mode] = _build(mask_mode)
    return _CACHED[mask_mode]


def _make_in_maps(x, attention_mask, position_ids, Wq, Wk, Wv, Wo, mask_mode):
    x = np.asarray(x, dtype=np.float32)
    assert x.shape == (B, S, H), x.shape
    attention_mask = np.asarray(attention_mask, dtype=np.float32)
    position_ids = np.asarray(position_ids)
    Wq = np.asarray(Wq, dtype=np.float32)
    Wk = np.asarray(Wk, dtype=np.float32)
    Wv = np.asarray(Wv, dtype=np.float32)
    Wo = np.asarray(Wo, dtype=np.float32)

    in_maps = []
    for c in range(8):
        b, l = c // TP, c % TP
        pos = position_ids[b].astype(np.float32) * 0.01
        mid = 0.5 * (pos.max() + pos.min())
        posq_b = np.ascontiguousarray(
            np.broadcast_to((pos / np.sqrt(HD))[None, :], (128, S))).astype(np.float32)
        posk_b = np.ascontiguousarray(
            np.broadcast_to((pos - mid)[None, :], (128, S))).astype(np.float32)

        # Wq rows for this core, re-tiled head-major:
        # wqt[d, hh, hc, do] = Wq[512l + hh*128 + do, hc*128 + d]
        wq_sl = Wq[OSL * l:OSL * (l + 1), :]
        wqt_c = np.ascontiguousarray(
            wq_sl.reshape(QH, HD, NHC, 128).transpose(3, 0, 2, 1)
        ).astype(ml_dtypes.bfloat16)

        # Wo columns permuted to the gathered order: block a=h, rank lr ->
        # global head 4*lr + h
        wo_sl = Wo[OSL * l:OSL * (l + 1), :]                       # [512, H]
        cols = [wo_sl[:, (4 * lr + h) * HD:(4 * lr + h + 1) * HD]
                for h in range(QH) for lr in range(TP)]
        wo_perm = np.concatenate(cols, axis=1)                     # [512, H]

        maskb_b = (-1e9 * (1.0 - attention_mask[b]))[None, :].astype(np.float32)

        in_maps.append({
            "xt": np.ascontiguousarray(x[b].T).astype(ml_dtypes.bfloat16),
            "wqt": wqt_c,
            "wkt": np.ascontiguousarray(
                Wk[HD * l:HD * (l + 1), :].T.reshape(NHC, 128, HD)
                .transpose(1, 0, 2)).astype(ml_dtypes.bfloat16),
            "wvt": np.ascontiguousarray(
                Wv[HD * l:HD * (l + 1), :].T.reshape(NHC, 128, HD)
                .transpose(1, 0, 2)).astype(ml_dtypes.bfloat16),
            "wot": np.ascontiguousarray(
                wo_perm.T.reshape(NHC, 128, OSL).transpose(1, 0, 2)
            ).astype(ml_dtypes.bfloat16),
            "posq": posq_b,
            "posk": posk_b,
            "maskb": np.ascontiguousarray(maskb_b),
        })
    return in_maps


def _run(x, attention_mask, position_ids, Wq, Wk, Wv, Wo, trace=False):
    am = np.asarray(attention_mask, dtype=np.float32)
    if np.all(am == 1.0):
        mask_mode = "ones"
    elif np.all((am == 0.0) | (am == 1.0)):
        mask_mode = "binary"
    else:
        mask_mode = "binary"  # fractional masks unsupported exactly; best effort

    nc = _get_nc(mask_mode)
    in_maps = _make_in_maps(x, attention_mask, position_ids, Wq, Wk, Wv, Wo,
                            mask_mode)
    res = run_bass_kernel_spmd(nc, in_maps, core_ids=list(range(8)),
                               trace=trace)
    y = np.empty((B, S, H), dtype=np.float32)
    for c in range(8):
        b, l = c // TP, c % TP
        y[b][:, OSL * l:OSL * (l + 1)] = res.results[c]["out"].T
    return y, res


def kernel(**inputs):
    y, _ = _run(**inputs, trace=False)
    return y


def kernel_profiled(**inputs):
    y, res = _run(**inputs, trace=True)
    return y, res


# revision 24
# speedup vs baseline: 1.1240x; 1.1240x over previous
"""GQA attention block (B=2, S=2048, H=2048, NH=16, NKV=4, HD=128) on 8 TRN2
NeuronCores.

Sharding: 2 batch groups x 4-way tensor parallel over heads.
Core c = b*4 + l handles batch b, q-heads [4l, 4l+4), kv-head l, and computes
output rows [512l, 512(l+1)) of y[b]^T after per-(head, i-half) AllGathers of
the context over its 4-core group. The host passes x and all weights
pre-transposed/pre-tiled (pure layout; bf16 where unit-scale) so the device
does no layout work on x/W at all. The output is produced transposed
([OSL, S]); the host transposes back for free at unshard time.

Per-core device pipeline (PE-queue kept gapless; consecutive matmuls share
lhsT wherever possible to skip the ~128-row weight reload):
  phase 1: three passes (k,v), (q0,q1), (q2,q3) over resident x tiles; each
           weight's 4x N=512 matmuls run back-to-back (one reload per weight
           per chunk). Epilogues add position bias split across DVE/Pool
           halves; vT staged bf16 via ACT.
  phase 2: isl-major software pipeline over 16 (islice, head) blocks; per
           128-query slot: S in two 2-bank PSUM halves (4x N=512 f32r sharing
           the qt lhsT), row-max from a stride-16 subset of the second half
           only (safe: margin analysis in-line), two exps with fused bias +
           row-sum (ACT, bf16 out), deferred recip+normalize (DVE; the last
           slot's normalize is quarter-split so it never gates the PE).
           The previous block's A.T transposes and A@V (bf16, N=512) are
           emitted alternating after each slot with a one-quarter lag so the
           PE never waits on the Pool AT copybacks.
  comm:    AllGather of ctx.T (bf16) per (head, i-half) (8 x 0.5MB), fired
           as soon as both islices of the half are written; gathered halves
           are pulled on-chip immediately (sync queue).
  phase 3: y^T[o, i] = sum_hd woT[hd, o] ctx^T[hd, i]; per (o-chunk, hd-chunk)
           the f0/f1 half matmuls share the wo lhsT. Heads are accumulated in
           AllGather-completion order so the last AG hides under earlier work.

Numerics: f32r (~1e-4) for q/k + scores keeps the huge position-bias score
component (pos_i*pos_j*HD ~ 4.7e3 in logits) accurate; the k-side bias is
centered (c'_j = 0.01*pos_j - mid, softmax-invariant) to halve its rounding
noise; bf16 only where unit-scale (QKV inputs, attention weights, ctx, Wo).
Subset-max margin ('ones' mask): positions are nondecreasing, so the row max
sits in the top i-half up to the +-51-ish data spread (c'*|q.1|/sqrt(HD));
stride-16 with offset 15 includes j=S-1, leaving exp(S-m) <= ~e^63 and row
sums <= ~e^71, both comfortably inside f32/bf16 range.
"""
import numpy as np

import concourse.bass as bass
import concourse.mybir as mybir
from concourse import bacc, tile
from concourse.bass_utils import run_bass_kernel_spmd

import ml_dtypes

F32 = mybir.dt.float32
F32R = mybir.dt.float32r
BF16 = mybir.dt.bfloat16
AF = mybir.ActivationFunctionType
ALU = mybir.AluOpType

B, S, H = 2, 2048, 2048
NH, NKV, HD = 16, 4, 128
TP = 4                      # tensor-parallel group size
QH = NH // TP               # q heads per core (4)
OSL = H // TP               # output rows per core (512)
SCALE = 1.0 / np.sqrt(HD)
NHC = H // 128              # 16 contraction chunks of 128
NIT = S // 128              # 16 i-tiles
NISL = S // 512             # 4 i-slices of 512
SH = S // 2                 # i-half (1024)

_CACHED = {}


def _build(mask_mode):
    """mask_mode: 'ones' (ignore mask) or 'binary' (additive -1e9 bias)."""
    nc = bacc.Bacc("TRN2", target_bir_lowering=False, debug=False, num_devices=8)

    xt = nc.dram_tensor("xt", [H, S], BF16, kind="ExternalInput")
    wqt = nc.dram_tensor("wqt", [128, QH, NHC, HD], BF16, kind="ExternalInput")
    wkt = nc.dram_tensor("wkt", [128, NHC, HD], BF16, kind="ExternalInput")
    wvt = nc.dram_tensor("wvt", [128, NHC, HD], BF16, kind="ExternalInput")
    wot = nc.dram_tensor("wot", [128, NHC, OSL], BF16, kind="ExternalInput")
    posq = nc.dram_tensor("posq", [128, S], F32, kind="ExternalInput")
    posk = nc.dram_tensor("posk", [128, S], F32, kind="ExternalInput")
    maskb = nc.dram_tensor("maskb", [1, S], F32, kind="ExternalInput")
    out = nc.dram_tensor("out", [OSL, S], F32, kind="ExternalOutput")

    idb_dram = nc.inline_tensor(np.eye(128, dtype=ml_dtypes.bfloat16), name="idb")

    groups = [[0, 1, 2, 3], [4, 5, 6, 7]]

    with tile.TileContext(nc) as tc:
        with (
            tc.tile_pool(name="pers", bufs=1) as pers,
            tc.tile_pool(name="small", bufs=14) as small,
            tc.tile_pool(name="dram", bufs=1, space="DRAM") as dram,
        ):
            # ---------------- persistent tiles ----------------
            qt_sb = pers.tile([128, QH, S], F32R)       # [d, h, i]  32KB/part
            kt_sb = pers.tile([128, S], F32R)           # [d, j]      8KB/part
            v_sb = pers.tile([128, NHC, HD], BF16)      # [j, jc, d]  4KB/part
            wo_sb = pers.tile([128, NHC, OSL], BF16)    # 16KB/part
            idb_sb = pers.tile([128, 128], BF16)

            # AG bounce buffers (per head, per i-half)
            cin = [[dram.tile([128, SH], BF16, name=f"cin{h}_{f}")
                    for f in range(2)] for h in range(QH)]
            gout = [[dram.tile([TP * 128, SH], BF16, name=f"gout{h}_{f}")
                     for f in range(2)] for h in range(QH)]

            # ---------------- phase 1: QKV projections ----------------
            with tc.tile_pool(name="p1w", bufs=1) as p1w:
                xt_sb = p1w.tile([128, NHC, S], BF16)    # resident x, 64KB/part
                wq_sb = p1w.tile([128, QH, NHC, HD], BF16)
                wk_sb = p1w.tile([128, NHC, HD], BF16)
                wv_sb = p1w.tile([128, NHC, HD], BF16)
                posq_sb = p1w.tile([128, S], F32)
                posk_sb = p1w.tile([128, S], F32)
                vt_stage = p1w.tile([128, S], BF16)      # vT [d, j] staged

                # pass-1 gates on only wk+wv+xt[0] (~1.6MB); wq/pos/wo are
                # dispatched after the pass-1 matmul emission so their
                # transfers don't steal HBM bandwidth from the gating loads.
                nc.scalar.dma_start(wk_sb[:], wkt[:])
                nc.scalar.dma_start(wv_sb[:], wvt[:])
                for hc in range(NHC):
                    nc.sync.dma_start(xt_sb[:, hc, :],
                                      xt[hc * 128:(hc + 1) * 128, :])
                nc.sync.dma_start(idb_sb[:], idb_dram.ap())

                def epi_add(dst, src, scalar, bias):
                    # DVE only: Pool/GpSimd cannot read PSUM on TRN2
                    for half in range(2):
                        sl = slice(half * SH, (half + 1) * SH)
                        nc.vector.scalar_tensor_tensor(
                            dst[:, sl], src[:, sl], scalar, bias[:, sl],
                            op0=ALU.mult, op1=ALU.add)

                with tc.tile_pool(name="p1ps", bufs=1, space="PSUM") as p1ps:
                    def pass_mms(wa, wb, pa, pb):
                        # matmul out is capped at one PSUM bank (512 f32):
                        # 4 slices back-to-back sharing the lhsT
                        for hc in range(NHC):
                            st, sp = hc == 0, hc == NHC - 1
                            for w_ap, pt in ((wa, pa), (wb, pb)):
                                for q4 in range(4):
                                    cs = slice(q4 * 512, (q4 + 1) * 512)
                                    nc.tensor.matmul(
                                        pt[:, cs], w_ap(hc), xt_sb[:, hc, cs],
                                        start=st, stop=sp,
                                        skip_group_check=True)

                    # pass 1: k, v
                    kp = p1ps.tile([128, S], F32, tag="pa", name="kp")
                    vp = p1ps.tile([128, S], F32, tag="pb", name="vp")
                    pass_mms(lambda hc: wk_sb[:, hc, :],
                             lambda hc: wv_sb[:, hc, :], kp, vp)
                    # non-gating loads dispatched once pass-1 owns the HBM
                    for hh in range(QH):
                        nc.scalar.dma_start(wq_sb[:, hh], wqt[:, hh])
                    nc.gpsimd.dma_start(posk_sb[:], posk[:])
                    nc.gpsimd.dma_start(posq_sb[:], posq[:])
                    epi_add(kt_sb, kp, 1.0, posk_sb)
                    nc.scalar.copy(vt_stage[:, 0:SH], vp[:, 0:SH])
                    nc.scalar.copy(vt_stage[:, SH:S], vp[:, SH:S])

                    # passes 2-3: q pairs
                    for pair in ((0, 1), (2, 3)):
                        qp = {hh: p1ps.tile([128, S], F32, tag=t,
                                            name=f"qp{hh}")
                              for hh, t in zip(pair, ("pa", "pb"))}
                        pass_mms(lambda hc, a=pair[0]: wq_sb[:, a, hc, :],
                                 lambda hc, b=pair[1]: wq_sb[:, b, hc, :],
                                 qp[pair[0]], qp[pair[1]])
                        for hh in pair:
                            epi_add(qt_sb[:, hh, :], qp[hh], SCALE, posq_sb)

            # mask bias (binary mode): additive row [1, S] f32r for accum-MM
            if mask_mode == "binary":
                maskb_sb = pers.tile([1, S], F32R)
                ones1_sb = pers.tile([1, 128], F32R)
                nc.gpsimd.dma_start(maskb_sb[:], maskb[:])
                ones_dram = nc.inline_tensor(
                    np.ones((1, 128), dtype=np.float32), name="ones1")
                nc.gpsimd.dma_start(ones1_sb[:], ones_dram.ap())

            # wo prefetch (phase 3), behind pos on the same queue
            nc.gpsimd.dma_start(wo_sb[:], wot[:])

            # ---------------- phase 2: attention ----------------
            cfp_cm = tc.tile_pool(name="cfp", bufs=1)
            cfp = cfp_cm.__enter__()
            ctxf = [[cfp.tile([128, TP, SH], BF16, name=f"ctxf{f}_{a}")
                     for f in range(2)] for a in range(QH)]  # 64KB/part
            with (
                tc.tile_pool(name="p2s", bufs=1, space="PSUM") as p2s,
                tc.tile_pool(name="p2tp", bufs=2, space="PSUM") as p2tp,
                tc.tile_pool(name="p2cx", bufs=2, space="PSUM") as p2cx,
                tc.tile_pool(name="p2a", bufs=8) as p2a,
                tc.tile_pool(name="p2at", bufs=3) as p2at,
                tc.tile_pool(name="p2ctx", bufs=1) as p2ctx,
            ):
                ctxT = {(h, f): p2ctx.tile([128, SH], BF16, name=f"cT{h}_{f}")
                        for h in range(QH) for f in range(2)}

                def scores_slot(blk, it4):
                    h, isl = blk["h"], blk["isl"]
                    it = isl * 4 + it4
                    Shs = [p2s.tile([128, SH], F32, tag=f"S{x}",
                                    name=f"S{x}_{h}_{it}") for x in range(2)]
                    q_ap = qt_sb[:, h, it * 128:(it + 1) * 128]
                    for js in range(4):
                        Sx = Shs[js // 2]
                        cs = slice((js % 2) * 512, (js % 2) * 512 + 512)
                        nc.tensor.matmul(
                            Sx[:, cs], q_ap, kt_sb[:, js * 512:(js + 1) * 512],
                            start=True, stop=(mask_mode != "binary"),
                            skip_group_check=True)
                        if mask_mode == "binary":
                            nc.tensor.matmul(
                                Sx[:, cs], ones1_sb[:],
                                maskb_sb[:, js * 512:(js + 1) * 512],
                                start=False, stop=True, skip_group_check=True)
                    negm = small.tile([128, 1], F32, tag="nm",
                                      name=f"nm_{h}_{it}")
                    if mask_mode == "binary":
                        # full-row subset max: either half may be masked out
                        n0 = small.tile([128, 1], F32, tag="n0", name=f"n0_{it}")
                        nc.vector.tensor_reduce(
                            n0[:], Shs[0][:, 15:SH:16],
                            axis=mybir.AxisListType.X, op=ALU.max, negate=True)
                        nc.vector.tensor_reduce(
                            negm[:], Shs[1][:, 15:SH:16],
                            axis=mybir.AxisListType.X, op=ALU.max, negate=True)
                        nc.vector.scalar_tensor_tensor(
                            negm[:], negm[:], 0.0, n0[:],
                            op0=ALU.add, op1=ALU.min)
                    else:
                        # top-half subset max only (margin analysis in module
                        # docstring: positions nondecreasing => safe shift)
                        nc.vector.tensor_reduce(
                            negm[:], Shs[1][:, 15:SH:16],
                            axis=mybir.AxisListType.X, op=ALU.max, negate=True)
                    A = p2a.tile([128, S], BF16, tag="A", name=f"A_{h}_{it}")
                    sums = [small.tile([128, 1], F32, tag=f"su{x}",
                                       name=f"su{x}_{h}_{it}") for x in range(2)]
                    for x in range(2):
                        nc.scalar.activation(
                            A[:, x * SH:(x + 1) * SH], Shs[x][:], AF.Exp,
                            bias=negm[:], scale=1.0, accum_out=sums[x][:])
                    return {"A": A, "sums": sums, "it": it, "r": None}

                def make_r(slot):
                    st = small.tile([128, 1], F32, tag="st",
                                    name=f"st_{slot['it']}")
                    r = small.tile([128, 1], F32, tag="r",
                                   name=f"r_{slot['it']}")
                    nc.vector.scalar_tensor_tensor(
                        st[:], slot["sums"][0][:], 0.0, slot["sums"][1][:],
                        op0=ALU.add, op1=ALU.add)
                    nc.vector.reciprocal(r[:], st[:])
                    slot["r"] = r

                def emit_norm_full(slot):
                    # DVE: bf16 in/out runs ~2.7 elem/cycle; GpSimd is a
                    # software DSP (~15 G elem/s) -- never bulk work there
                    make_r(slot)
                    nc.vector.tensor_scalar_mul(
                        slot["A"][:], slot["A"][:], slot["r"][:])

                def emit_norm3_quarter(blk, q):
                    # make_r(slot3) was emitted at the end of its own block,
                    # so only the quarter multiply sits before the T-group
                    slot = blk["slots"][3]
                    cs = slice(q * 512, (q + 1) * 512)
                    nc.vector.tensor_scalar_mul(
                        slot["A"][:, cs], slot["A"][:, cs], slot["r"][:])

                def emit_tgroup(blk, jc):
                    it4 = jc // 4
                    if jc % 4 == 0:
                        blk["atq"][it4] = p2at.tile(
                            [128, 4, 512], BF16, tag="AT",
                            name=f"AT{blk['bi']}_{it4}")
                    atq = blk["atq"][it4]
                    tp = p2tp.tile([128, 512], BF16, tag="tp",
                                   name=f"tp{blk['bi']}_{jc}")
                    for k in range(4):
                        nc.tensor.transpose(
                            tp[:, k * 128:(k + 1) * 128],
                            blk["slots"][k]["A"][:, jc * 128:(jc + 1) * 128],
                            idb_sb[:])
                    # PSUM->SBUF copybacks all on DVE: ACT's two exps per
                    # slot already run near the slot period
                    nc.vector.tensor_copy(atq[:, jc % 4, :], tp[:])

                def emit_av(blk, jc):
                    nc.tensor.matmul(
                        blk["ctxp"][:], v_sb[:, jc, :],
                        blk["atq"][jc // 4][:, jc % 4, :],
                        start=(jc == 0), stop=(jc == NHC - 1))
                    if jc == NHC - 1:
                        finish_block(blk)

                def finish_block(blk):
                    h, isl = blk["h"], blk["isl"]
                    f, seg = isl // 2, isl % 2
                    ct = ctxT[(h, f)]
                    nc.vector.tensor_copy(
                        ct[:, seg * 512:(seg + 1) * 512], blk["ctxp"][:])
                    if seg == 1:
                        nc.sync.dma_start(cin[h][f][:], ct[:])
                        nc.gpsimd.collective_compute(
                            "AllGather", ALU.bypass,
                            ins=[cin[h][f][:].opt()],
                            outs=[gout[h][f][:].opt()],
                            replica_groups=groups)
                        nc.sync.dma_start(
                            ctxf[h][f][:],
                            gout[h][f][:].rearrange("(lr p) i -> p lr i", p=128))

                # h-major: head h's second-half AllGather fires at block
                # 4h+4, so h0-h2's f1 data is on-chip long before phase 3;
                # only h3's last AG rides on the ~44us of f0+f1(a<3) work
                blocks = [(isl, h) for h in range(QH) for isl in range(NISL)]
                prev = None
                tq = []        # pending (blk, jc) whose AV trails 4 T-groups
                vt_done = 0    # v-transpose groups used to fill block 0
                for bi, (isl, h) in enumerate(blocks):
                    blk = {"h": h, "isl": isl, "bi": bi, "slots": [],
                           "atq": {},
                           "ctxp": p2cx.tile([128, 512], F32, tag="cx",
                                             name=f"cx{bi}")}
                    for it4 in range(4):
                        if prev is not None:
                            emit_norm3_quarter(prev, it4)
                        blk["slots"].append(scores_slot(blk, it4))
                        if it4 >= 1:
                            emit_norm_full(blk["slots"][it4 - 1])
                        for k in range(4):
                            # AV first: its copyback landed ~4 groups ago, so
                            # it never waits; the T-group behind it gives the
                            # norm3 chain time at block starts
                            if len(tq) >= 4:
                                emit_av(*tq.pop(0))
                            if prev is not None:
                                emit_tgroup(prev, it4 * 4 + k)
                                tq.append((prev, it4 * 4 + k))
                            elif vt_done < 4 and k == 0:
                                # fill block 0: 4 v-transposes per slot
                                g = vt_done
                                tpv = p2tp.tile([128, 512], BF16, tag="tp",
                                                name=f"tpv{g}")
                                for kk in range(4):
                                    jc = g * 4 + kk
                                    nc.tensor.transpose(
                                        tpv[:, kk * 128:(kk + 1) * 128],
                                        vt_stage[:, jc * 128:(jc + 1) * 128],
                                        idb_sb[:])
                                nc.vector.tensor_copy(
                                    v_sb[:, g * 4:(g + 1) * 4, :], tpv[:])
                                vt_done += 1
                    make_r(blk["slots"][3])
                    prev = blk
                # drain the last block
                for it4 in range(4):
                    emit_norm3_quarter(prev, it4)
                    for k in range(4):
                        if len(tq) >= 4:
                            emit_av(*tq.pop(0))
                        emit_tgroup(prev, it4 * 4 + k)
                        tq.append((prev, it4 * 4 + k))
                while tq:
                    emit_av(*tq.pop(0))

            # ---------------- phase 3: output projection (y^T) ----------
            with (
                tc.tile_pool(name="p3y", bufs=2, space="PSUM") as p3y,
                tc.tile_pool(name="p3o", bufs=3) as p3o,
            ):
                # f0 pass first: its AllGathers completed mid-phase-2, so the
                # PE rolls straight in; the last f1 AllGather (fired at the
                # end of phase 2) hides under the ~30us of f0 work.
                for f in range(2):
                    for o in range(OSL // 128):
                        yp = p3y.tile([128, SH], F32, tag=f"y{o % 2}",
                                      name=f"yp{o}_{f}")
                        for cc in range(NHC):
                            a, lr = cc // 4, cc % 4
                            w_ap = wo_sb[:, cc, o * 128:(o + 1) * 128]
                            st, sp = cc == 0, cc == NHC - 1
                            for q2 in range(2):
                                cs = slice(q2 * 512, (q2 + 1) * 512)
                                nc.tensor.matmul(
                                    yp[:, cs], w_ap, ctxf[a][f][:, lr, cs],
                                    start=st, stop=sp, skip_group_check=True)
                        y_sb = p3o.tile([128, SH], F32, tag="y_sb",
                                        name=f"y_sb{o}_{f}")
                        if o % 2 == 0:
                            nc.vector.tensor_copy(y_sb[:], yp[:])
                        else:
                            nc.scalar.copy(y_sb[:], yp[:])
                        nc.sync.dma_start(
                            out[o * 128:(o + 1) * 128, f * SH:(f + 1) * SH],
                            y_sb[:])

            cfp_cm.__exit__(None, None, None)

    nc.compile()
    return nc


def _get_nc(mask_mode):
    if mask_mode not in _CACHED:
        _CACHED[mask_mode] = _build(mask_mode)
    return _CACHED[mask_mode]


def _make_in_maps(x, attention_mask, position_ids, Wq, Wk, Wv, Wo, mask_mode):
    x = np.asarray(x, dtype=np.float32)
    assert x.shape == (B, S, H), x.shape
    attention_mask = np.asarray(attention_mask, dtype=np.float32)
    position_ids = np.asarray(position_ids)
    Wq = np.asarray(Wq, dtype=np.float32)
    Wk = np.asarray(Wk, dtype=np.float32)
    Wv = np.asarray(Wv, dtype=np.float32)
    Wo = np.asarray(Wo, dtype=np.float32)

    in_maps = []
    for c in range(8):
        b, l = c // TP, c % TP
        pos = position_ids[b].astype(np.float32) * 0.01
        mid = 0.5 * (pos.max() + pos.min())
        posq_b = np.ascontiguousarray(
            np.broadcast_to((pos / np.sqrt(HD))[None, :], (128, S))).astype(np.float32)
        posk_b = np.ascontiguousarray(
            np.broadcast_to((pos - mid)[None, :], (128, S))).astype(np.float32)

        # Wq rows for this core, re-tiled head-major:
        # wqt[d, hh, hc, do] = Wq[512l + hh*128 + do, hc*128 + d]
        wq_sl = Wq[OSL * l:OSL * (l + 1), :]
        wqt_c = np.ascontiguousarray(
            wq_sl.reshape(QH, HD, NHC, 128).transpose(3, 0, 2, 1)
        ).astype(ml_dtypes.bfloat16)

        # Wo columns permuted to the gathered order: block a=h, rank lr ->
        # global head 4*lr + h
        wo_sl = Wo[OSL * l:OSL * (l + 1), :]                       # [512, H]
        cols = [wo_sl[:, (4 * lr + h) * HD:(4 * lr + h + 1) * HD]
                for h in range(QH) for lr in range(TP)]
        wo_perm = np.concatenate(cols, axis=1)                     # [512, H]

        maskb_b = (-1e9 * (1.0 - attention_mask[b]))[None, :].astype(np.float32)

        in_maps.append({
            "xt": np.ascontiguousarray(x[b].T).astype(ml_dtypes.bfloat16),
            "wqt": wqt_c,
            "wkt": np.ascontiguousarray(
                Wk[HD * l:HD * (l + 1), :].T.reshape(NHC, 128, HD)
                .transpose(1, 0, 2)).astype(ml_dtypes.bfloat16),
            "wvt": np.ascontiguousarray(
                Wv[HD * l:HD * (l + 1), :].T.reshape(NHC, 128, HD)
                .transpose(1, 0, 2)).astype(ml_dtypes.bfloat16),
            "wot": np.ascontiguousarray(
                wo_perm.T.reshape(NHC, 128, OSL).transpose(1, 0, 2)
            ).astype(ml_dtypes.bfloat16),
            "posq": posq_b,
            "posk": posk_b,
            "maskb": np.ascontiguousarray(maskb_b),
        })
    return in_maps


def _run(x, attention_mask, position_ids, Wq, Wk, Wv, Wo, trace=False):
    am = np.asarray(attention_mask, dtype=np.float32)
    if np.all(am == 1.0):
        mask_mode = "ones"
    elif np.all((am == 0.0) | (am == 1.0)):
        mask_mode = "binary"
    else:
        mask_mode = "binary"  # fractional masks unsupported exactly; best effort

    nc = _get_nc(mask_mode)
    in_maps = _make_in_maps(x, attention_mask, position_ids, Wq, Wk, Wv, Wo,
                            mask_mode)
    res = run_bass_kernel_spmd(nc, in_maps, core_ids=list(range(8)),
                               trace=trace)
    y = np.empty((B, S, H), dtype=np.float32)
    for c in range(8):
        b, l = c // TP, c % TP
        y[b][:, OSL * l:OSL * (l + 1)] = res.results[c]["out"].T
    return y, res


def kernel(**inputs):
    y, _ = _run(**inputs, trace=False)
    return y


def kernel_profiled(**inputs):
    y, res = _run(**inputs, trace=True)
    return y, res
